# revision 36
# baseline (speedup 1.0000x reference)
"""CharCNN embedder (ELMo-style) Trainium2 Bass kernel, v3.

Strategy (pure data parallel over 8 cores, 256 tokens each):
  - Embedding lookup as one-hot matmul: one-hot encodings of the char ids
    arrive pre-built from the host (3 x [128, COLS_PAD] bf16, exact 0/1);
    the device runs embT.T @ onehot over 3 row-chunks of the 262-entry
    table -> xT [16, 12800] directly in conv layout. The emb chunks are
    interleaved into the conv stream so they fill PE gaps while the DVE
    drains conv PSUM.
  - im2col by 6 shifted SBUF->SBUF DMA copies -> X7 [112+6, 12800+pad].
    Rows 112..117 are per-position-class indicator rows; the conv weight
    matrix carries -1e30 in those rows for (channel, position) pairs that
    are invalid, so max-over-time needs no masking pass.
  - All 7 convs as one packed [118, 2048] bf16 matmul into 2-bank PSUM
    tiles (16 tokens each); max-over-time as one strided DVE reduce per
    tile (the DVE is the only engine that can drain PSUM with a max; its
    ~226us of reduce work is the kernel's critical path).
  - Highway + projection in token-major form overlapped under the reduce
    stream: layer-0 token-half 0 runs while the conv's second half is
    still reducing. Weight slabs streamed from DRAM in bf16.
"""

import os
import numpy as np
import ml_dtypes

import concourse.bass as bass
import concourse.mybir as mybir
import concourse.tile as tile
from concourse.bass_utils import run_bass_kernel_spmd

F32 = mybir.dt.float32
BF16 = mybir.dt.bfloat16
NPBF16 = ml_dtypes.bfloat16

CNN_OPTIONS = [(1, 32), (2, 32), (3, 64), (4, 128), (5, 256), (6, 512), (7, 1024)]
EMB_DIM = 16
N_CHARS = 262
MAX_CHARS = 50
N_FILTERS = 2048
OUT_DIM = 512
N_HIGHWAY = 2
BATCH, SEQ = 4, 512
NCORES = 8
T_LOC = BATCH * SEQ // NCORES          # 256 tokens per core
COLS = T_LOC * MAX_CHARS               # 12800
COLS_PAD = COLS + 16                   # 12816
KMAX = 7
KROWS = EMB_DIM * KMAX                 # 112
KTOT = KROWS + 6                       # 118 (6 indicator rows for pos 44..49)
NCH = 512                              # xT build chunk width
NXCH = COLS // NCH                     # 25
TOKG = 8                               # tokens per conv matmul
# oc-chunk list: (chunk idx -> kernel size driving its valid-position count)
CHUNK_K = [1, 4, 5, 5, 6, 6, 6, 6, 7, 7, 7, 7, 7, 7, 7, 7]  # m=0 mixed (use 50 pos)
CHUNK_NP = [50 if k == 1 else (MAX_CHARS - k + 1) for k in CHUNK_K]
HWM = 32                               # 4096/128 output chunks per highway layer
KC = 16                                # 2048/128 contraction chunks


def _split_multi_waits(nc):
    """This walrus build encodes at most ONE sync-wait per instruction.
    Hoist extra waits onto dedicated NoOps ahead of the instruction."""
    ctr = [0]
    for f in nc.m.functions:
        for b in f.blocks:
            il = b.instructions
            if not any(
                i.sync_info is not None and len(i.sync_info.on_wait) > 1 for i in il
            ):
                continue
            new = []
            for ins in il:
                si = ins.sync_info
                if si is not None and len(si.on_wait) > 1:
                    waits = list(si.on_wait)
                    for w in waits[:-1]:
                        ctr[0] += 1
                        nop = mybir.InstNoOp(name=f"wsplit-{ctr[0]}", ins=[], outs=[])
                        nop.engine = ins.engine
                        nop.sync_info = mybir.SyncInfo(on_wait=[w], on_update=[])
                        new.append(nop)
                    ins.sync_info = mybir.SyncInfo(
                        on_wait=[waits[-1]], on_update=list(si.on_update)
                    )
                new.append(ins)
            b.instructions = new


def _build_program(split_waits=True):
    nc = bass.Bass(target_bir_lowering=False)

    oh_d = [
        nc.dram_tensor(f"oh{r}", [128, COLS_PAD], BF16, kind="ExternalInput")
        for r in range(3)
    ]
    embt_d = nc.dram_tensor("embt", [384, EMB_DIM], BF16, kind="ExternalInput")
    convw_d = nc.dram_tensor("convw", [KTOT, N_FILTERS], BF16, kind="ExternalInput")
    indic_d = nc.dram_tensor("indic", [6, COLS_PAD], BF16, kind="ExternalInput")
    cbias_d = nc.dram_tensor("cbias", [128, 16], F32, kind="ExternalInput")
    hw0_d = nc.dram_tensor("hw0", [KC, 4, 128, 1024], BF16, kind="ExternalInput")
    hw1_d = nc.dram_tensor("hw1", [KC, 4, 128, 1024], BF16, kind="ExternalInput")
    hb0_d = nc.dram_tensor("hb0", [1, 4096], BF16, kind="ExternalInput")
    hb1_d = nc.dram_tensor("hb1", [1, 4096], BF16, kind="ExternalInput")
    pw_d = nc.dram_tensor("pw", [KC, 128, 512], BF16, kind="ExternalInput")
    pb_d = nc.dram_tensor("pb", [1, 512], BF16, kind="ExternalInput")
    out_d = nc.dram_tensor("outT", [T_LOC, OUT_DIM], F32, kind="ExternalOutput")

    with tile.TileContext(nc) as tc:
        with (
            tc.tile_pool(name="const", bufs=1) as cpool,
            tc.tile_pool(name="elem", bufs=3) as epool,
            tc.tile_pool(name="outp", bufs=2) as outpool,
            tc.tile_pool(name="ps_big", bufs=2, space="PSUM") as ps_big,
            tc.tile_pool(name="ps_xt", bufs=2, space="PSUM") as ps_xt,
            tc.tile_pool(name="ps_hw", bufs=2, space="PSUM") as ps_hw,
        ):
            # one-hot encodings stream per 512-col chunk (3 tiles each);
            # the ring lets the SP run several chunks ahead of the PE
            ohpool_cm = tc.tile_pool(name="ohp", bufs=8)
            ohpool = ohpool_cm.__enter__()

            # ---- constants in ----
            embt_s = cpool.tile([128, 3 * EMB_DIM], BF16, tag="embt")
            for r in range(3):
                nc.sync.dma_start(
                    embt_s[:, 16 * r : 16 * r + 16], embt_d[128 * r : 128 * r + 128, :]
                )
            cbias_s = cpool.tile([128, 16], F32, tag="cbias")
            nc.sync.dma_start(cbias_s[:], cbias_d[:])
            # X7: rows 0-15 xT base, 16-111 shifted copies, 112-117 indicators
            X7 = cpool.tile([KTOT, COLS_PAD], BF16, tag="X7")
            convw_s = cpool.tile([KTOT, N_FILTERS], BF16, tag="convw")
            hb0_s = cpool.tile([1, 4096], BF16, tag="hb0")
            nc.sync.dma_start(hb0_s[:], hb0_d[:])
            hb1_s = cpool.tile([1, 4096], BF16, tag="hb1")
            nc.sync.dma_start(hb1_s[:], hb1_d[:])
            pb_s = cpool.tile([1, 512], BF16, tag="pb")
            nc.sync.dma_start(pb_s[:], pb_d[:])
            onesb_s = cpool.tile([1, 128], BF16, tag="onesb")
            nc.gpsimd.memset(onesb_s[:], 1.0)
            from concourse.masks import make_identity
            ident_s = cpool.tile([128, 128], BF16, tag="ident")
            make_identity(nc, ident_s[:])

            hts = [cpool.tile([128, T_LOC], BF16, tag=f"hT{m}", name=f"hT{m}") for m in range(KC)]
            h1ts = [cpool.tile([128, T_LOC], BF16, tag=f"h1T{m}", name=f"h1T{m}") for m in range(KC)]
            h2ts = [cpool.tile([128, T_LOC], BF16, tag=f"h2T{m}", name=f"h2T{m}") for m in range(KC)]
            htok = [cpool.tile([128, N_FILTERS], BF16, tag=f"htok{t}", name=f"htok{t}") for t in range(2)]
            h1tok = [cpool.tile([128, N_FILTERS], BF16, tag=f"h1tok{t}", name=f"h1tok{t}") for t in range(2)]

            # ---- helpers ----
            def transpose_block(src_ap, dst_ap, nm):
                tp = ps_xt.tile([128, 128], BF16, space="PSUM", tag="xt", name=nm)
                nc.tensor.transpose(tp[:], src_ap, ident_s[:])
                nc.scalar.copy(dst_ap, tp[:])

            # emb chunk: stream 3 one-hot chunk tiles in, 3 accumulated
            # matmuls -> X7 rows 0:16
            def emb_chunk(c):
                c0 = c * NCH
                ohs = []
                for r in range(3):
                    t = ohpool.tile([128, NCH], BF16, tag=f"oh{r}", name=f"oh{r}_{c}")
                    nc.scalar.dma_start(t[:], oh_d[r][:, c0 : c0 + NCH])
                    ohs.append(t)
                px = ps_xt.tile([16, NCH], F32, space="PSUM", tag="xt", name=f"px{c}")
                for r in range(3):
                    nc.tensor.matmul(
                        px[:],
                        embt_s[:, 16 * r : 16 * r + 16],
                        ohs[r][:],
                        start=(r == 0),
                        stop=(r == 2),
                    )
                nc.scalar.copy(X7[0:16, c0 : c0 + NCH], px[:])

            def shifts(cl, cu):
                for j in range(1, KMAX):
                    nc.sync.dma_start(
                        X7[16 * j : 16 * j + 16, cl:cu], X7[0:16, cl + j : cu + j]
                    )

            # conv unit: 16 tokens x one 128-chan chunk, one strided reduce
            def conv_unit(nnp, m):
                npos = CHUNK_NP[m]
                ps = ps_big.tile(
                    [128, 1024], F32, space="PSUM", tag="big", name=f"cv{m}_{nnp}"
                )
                for h in range(2):
                    c0 = (2 * nnp + h) * TOKG * MAX_CHARS
                    rhs = (
                        X7[0:KTOT, c0 : c0 + TOKG * MAX_CHARS]
                        .rearrange("p (t c) -> p t c", c=MAX_CHARS)[:, :, 0:npos]
                    )
                    nc.tensor.matmul(
                        ps[:, 512 * h : 512 * h + TOKG * npos],
                        convw_s[:, 128 * m : 128 * m + 128],
                        rhs,
                        start=True,
                        stop=True,
                    )
                ps4 = (
                    ps[:]
                    .rearrange("p (h x) -> p h x", h=2)[:, :, 0 : TOKG * npos]
                    .rearrange("p h (t c) -> p h t c", c=npos)
                )
                dst = hts[m][:, 16 * nnp : 16 * nnp + 16].rearrange(
                    "p (h t) -> p h t", h=2
                )
                nc.vector.reduce_max(dst, ps4, axis=mybir.AxisListType.X)

            def finish_half(half):
                for m in range(16):
                    hsl = hts[m][:, 128 * half : 128 * half + 128]
                    nc.scalar.activation(
                        hsl, hsl, mybir.ActivationFunctionType.Relu,
                        bias=cbias_s[:, m : m + 1], scale=1.0,
                    )
                for c in range(KC):
                    transpose_block(
                        hts[c][:, 128 * half : 128 * half + 128],
                        htok[half][:, 128 * c : 128 * c + 128],
                        f"tpc{half}_{c}",
                    )

            # one DMA per (layer, g, k-half): 8 k-slabs land in a single
            # [128, 8*1024] group tile (8x fewer SP issues and sem hops)
            def hw_grp_dma(gpool, lyr, g, mt):
                w_d = hw0_d if lyr == 0 else hw1_d
                grps = []
                for hk in range(2):
                    grp = gpool.tile(
                        [128, 8 * 1024], BF16, tag="wgrp",
                        name=f"wg{lyr}{g}{mt}{hk}",
                    )
                    nc.sync.dma_start(
                        grp[:].rearrange("p (k x) -> p k x", k=8),
                        w_d[8 * hk : 8 * hk + 8, g].rearrange("k p x -> p k x"),
                    )
                    grps.append(grp)
                return grps

            def hw_chain(lyr, g, bb, mt, grp):
                src_ts = hts if lyr == 0 else h1ts
                h_in = htok if lyr == 0 else h1tok
                hb_s = hb0_s if lyr == 0 else hb1_s
                b = 2 * g + bb
                ps = ps_hw.tile(
                    [128, 512], F32, space="PSUM", tag="hwp",
                    name=f"hwp{lyr}{b}{mt}",
                )
                for k in range(KC):
                    gh = grp[k // 8]
                    o = 1024 * (k % 8) + 512 * bb
                    nc.tensor.matmul(
                        ps[:],
                        src_ts[k][:, 128 * mt : 128 * mt + 128],
                        gh[:, o : o + 512],
                        start=(k == 0), stop=False,
                    )
                nc.tensor.matmul(
                    ps[:], onesb_s[0:1, :],
                    hb_s[0:1, 512 * b : 512 * b + 512],
                    start=False, stop=True,
                )
                r_ = epool.tile([128, 256], BF16, tag="relu", name=f"r{lyr}{b}{mt}")
                nc.scalar.activation(
                    r_[:], ps[:, 0:256], mybir.ActivationFunctionType.Relu
                )
                g_ = epool.tile([128, 256], BF16, tag="gate", name=f"g{lyr}{b}{mt}")
                nc.scalar.activation(
                    g_[:], ps[:, 256:512], mybir.ActivationFunctionType.Sigmoid
                )
                hsl = h_in[mt][:, 256 * b : 256 * b + 256]
                t1 = epool.tile([128, 256], BF16, tag="t1", name=f"t1_{lyr}{b}{mt}")
                nc.vector.tensor_tensor(
                    out=t1[:], in0=hsl, in1=r_[:], op=mybir.AluOpType.subtract
                )
                t2 = epool.tile([128, 256], BF16, tag="t2", name=f"t2_{lyr}{b}{mt}")
                nc.vector.tensor_tensor(
                    out=t2[:], in0=g_[:], in1=t1[:], op=mybir.AluOpType.mult
                )
                if lyr == 0:
                    dst_sl = h1tok[mt][:, 256 * b : 256 * b + 256]
                    nc.vector.tensor_tensor(
                        out=dst_sl, in0=t2[:], in1=r_[:], op=mybir.AluOpType.add
                    )
                    for c in (2 * b, 2 * b + 1):
                        transpose_block(
                            h1tok[mt][:, 128 * c : 128 * c + 128],
                            h1ts[c][:, 128 * mt : 128 * mt + 128],
                            f"tp1_{b}{mt}{c}",
                        )
                else:
                    h2sl = epool.tile(
                        [128, 256], BF16, tag="h2sl", name=f"h2sl{b}{mt}"
                    )
                    nc.vector.tensor_tensor(
                        out=h2sl[:], in0=t2[:], in1=r_[:], op=mybir.AluOpType.add
                    )
                    for ci, c in enumerate((2 * b, 2 * b + 1)):
                        transpose_block(
                            h2sl[:, 128 * ci : 128 * ci + 128],
                            h2ts[c][:, 128 * mt : 128 * mt + 128],
                            f"tp2_{b}{mt}{c}",
                        )

            # ---- pipelined schedule ----
            # emb chunks 0..8 unlock shift block 0 (cols 0..3578, reads to
            # 3583); conv token-half 0 (pairs 0..7) interleaves the remaining
            # emb chunks so the PE fills its reduce-paced gaps. Shift blocks
            # fire as soon as their emb chunks are done. The big const DMAs
            # (conv weights, indicators) are issued after the first few
            # one-hot chunks so the emb pipeline starts immediately.
            for c in range(3):
                emb_chunk(c)
            nc.sync.dma_start(convw_s[:], convw_d[:])
            nc.sync.dma_start(X7[112:118, :], indic_d[:])
            nc.gpsimd.memset(X7[0:16, COLS:COLS_PAD], 0.0)
            for c in range(3, 9):
                emb_chunk(c)
            shifts(0, 3578)
            next_c = [9]
            SH = {13: (3578, 6650), 19: (6650, 9722), 25: (9722, COLS + 6)}

            def pump_emb():
                if next_c[0] >= NXCH:
                    return
                c = next_c[0]
                emb_chunk(c)
                next_c[0] += 1
                if c + 1 in SH:
                    shifts(*SH[c + 1])

            u = 0
            for nnp in range(8):
                for m in range(16):
                    conv_unit(nnp, m)
                    u += 1
                    if u % 8 == 0:
                        pump_emb()
            while next_c[0] < NXCH:
                pump_emb()
            finish_half(0)

            with tc.tile_pool(name="wgrp", bufs=5) as gpool:
                # conv token-half 1 hosts the mt=0 highway work (both layers)
                # in its PE gaps; the mt=1 passes + projection trail it, with
                # their group DMAs streaming early thanks to the 3-deep ring.
                chains = [(0, g, bb) for g in range(4) for bb in range(2)] + [
                    (1, g, bb) for g in range(4) for bb in range(2)
                ]
                chains.append(("proj", 0, 0))
                grp_cache = {}
                pgrps = []

                def proj_mt(mt):
                    if not pgrps:
                        for hk in range(2):
                            pg = gpool.tile(
                                [128, 8 * 512], BF16, tag="wgrp", name=f"pgrp{hk}"
                            )
                            nc.sync.dma_start(
                                pg[:].rearrange("p (k x) -> p k x", k=8),
                                pw_d[8 * hk : 8 * hk + 8].rearrange("k p x -> p k x"),
                            )
                            pgrps.append(pg)
                    ps = ps_hw.tile(
                        [128, 512], F32, space="PSUM", tag="hwp", name=f"pj{mt}"
                    )
                    for k in range(KC):
                        nc.tensor.matmul(
                            ps[:],
                            h2ts[k][:, 128 * mt : 128 * mt + 128],
                            pgrps[k // 8][:, 512 * (k % 8) : 512 * (k % 8) + 512],
                            start=(k == 0), stop=False,
                        )
                    nc.tensor.matmul(
                        ps[:], onesb_s[0:1, :], pb_s[0:1, :], start=False, stop=True
                    )
                    oc = outpool.tile([128, 512], F32, tag="out", name=f"oc{mt}")
                    nc.scalar.copy(oc[:], ps[:])
                    nc.sync.dma_start(out_d[128 * mt : 128 * mt + 128, :], oc[:])

                def emit_chain(lyr, g, bb, mt):
                    if lyr == "proj":
                        proj_mt(mt)
                        return
                    key = (lyr, g, mt)
                    if key not in grp_cache:
                        grp_cache[key] = hw_grp_dma(gpool, lyr, g, mt)
                    hw_chain(lyr, g, bb, mt, grp_cache[key])

                u = 0
                for nnp in range(8, 16):
                    for m in range(16):
                        conv_unit(nnp, m)
                        u += 1
                        if u % 8 == 0 and chains:
                            emit_chain(*chains.pop(0), 0)
                for lyr, g, bb in chains:
                    emit_chain(lyr, g, bb, 0)
                finish_half(1)
                for g in range(4):
                    for bb in range(2):
                        emit_chain(0, g, bb, 1)
                for g in range(4):
                    for bb in range(2):
                        emit_chain(1, g, bb, 1)
                proj_mt(1)
            ohpool_cm.__exit__(None, None, None)

    if split_waits:
        _split_multi_waits(nc)
    return nc


def _prep_weights(inputs):
    conv_ws = [np.asarray(inputs[f"conv_w{i}"], np.float32) for i in range(7)]
    conv_bs = [np.asarray(inputs[f"conv_b{i}"], np.float32) for i in range(7)]

    W7 = np.zeros((KTOT, N_FILTERS), np.float32)
    o0 = 0
    for (ksz, oc), w in zip(CNN_OPTIONS, conv_ws):
        for j in range(ksz):
            W7[16 * j : 16 * j + 16, o0 : o0 + oc] = w[:, :, j].T
        # indicator-mask rows: position p = 44+i invalid iff p > 50-ksz
        for i in range(6):
            if (44 + i) > (MAX_CHARS - ksz):
                W7[KROWS + i, o0 : o0 + oc] = -1e30
        o0 += oc

    b_all = np.concatenate(conv_bs)
    cbias = b_all.reshape(16, 128).T.astype(np.float32)

    indic = np.zeros((6, COLS_PAD), np.float32)
    for i in range(6):
        indic[i, (44 + i) : COLS : MAX_CHARS] = 1.0

    emb = np.asarray(inputs["emb"], np.float32)
    embt = np.zeros((384, EMB_DIM), np.float32)
    embt[:N_CHARS] = emb

    # token-form highway: out = h @ w'.T computed as hT-chunks.T @ w'.T-slabs.
    # Channel-interleave so slab b holds [256 nonlin | 256 gate] of block b.
    perm = np.concatenate(
        [
            np.concatenate([np.arange(256 * b, 256 * b + 256),
                            2048 + np.arange(256 * b, 256 * b + 256)])
            for b in range(8)
        ]
    )

    def hw_slabs(w, bvec):
        wp = np.asarray(w, np.float32)[perm]       # [4096, 2048] permuted rows
        bp = np.asarray(bvec, np.float32)[perm]
        wt = wp.T                                   # [2048, 4096]
        arr = wt.reshape(KC, 128, 8, 512).transpose(0, 2, 1, 3)  # [16, 8, 128, 512]
        arr = arr.reshape(KC, 4, 2, 128, 512).transpose(0, 1, 3, 2, 4).reshape(KC, 4, 128, 1024)
        return np.ascontiguousarray(arr).astype(NPBF16), bp[None, :].astype(NPBF16)

    hw0, hb0 = hw_slabs(inputs["hw_w0"], inputs["hw_b0"])
    hw1, hb1 = hw_slabs(inputs["hw_w1"], inputs["hw_b1"])
    pwt = np.asarray(inputs["proj_w"], np.float32).T  # [2048, 512]
    pw = np.ascontiguousarray(pwt.reshape(KC, 128, 512)).astype(NPBF16)
    pb = np.asarray(inputs["proj_b"], np.float32)[None, :].astype(NPBF16)

    return {
        "embt": embt.astype(NPBF16),
        "convw": W7.astype(NPBF16),
        "indic": indic.astype(NPBF16),
        "cbias": cbias,
        "hw0": hw0.astype(NPBF16),
        "hw1": hw1.astype(NPBF16),
        "hb0": hb0,
        "hb1": hb1,
        "pw": pw.astype(NPBF16),
        "pb": pb,
    }


_NC_CACHE = []
LAST_RESULT = {}


def kernel(**inputs) -> np.ndarray:
    if not _NC_CACHE:
        _NC_CACHE.append(_build_program())
    nc = _NC_CACHE[0]

    shared = _prep_weights(inputs)
    ids = np.asarray(inputs["batch_ids"]).astype(np.int64).reshape(-1, MAX_CHARS)
    rng384 = np.arange(384, dtype=np.int64)
    in_maps = []
    for core in range(NCORES):
        flat = ids[core * T_LOC : (core + 1) * T_LOC].reshape(-1)
        oh = np.zeros((384, COLS_PAD), NPBF16)
        oh[:, :COLS] = (flat[None, :] == rng384[:, None]).astype(NPBF16)
        cm = {f"oh{r}": np.ascontiguousarray(oh[128 * r : 128 * r + 128]) for r in range(3)}
        in_maps.append({**cm, **shared})

    trace = bool(int(os.environ.get("KERNEL_TRACE", "0")))
    res = run_bass_kernel_spmd(
        nc, in_maps, core_ids=list(range(NCORES)), trace=trace
    )
    LAST_RESULT["exec_time_ns"] = res.exec_time_ns
    LAST_RESULT["trace"] = res.instructions_and_trace

    parts = [res.results[c]["outT"] for c in range(NCORES)]  # each [256, 512]
    out = np.concatenate(parts, axis=0).reshape(BATCH, SEQ, OUT_DIM)
    return np.ascontiguousarray(out.astype(np.float32))


# revision 37
# speedup vs baseline: 1.0519x; 1.0519x over previous
"""CharCNN embedder (ELMo-style) Trainium2 Bass kernel, v3.

Strategy (pure data parallel over 8 cores, 256 tokens each):
  - Embedding lookup as one-hot matmul: one-hot encodings of the char ids
    arrive pre-built from the host (3 x [128, COLS_PAD] bf16, exact 0/1);
    the device runs embT.T @ onehot over 3 row-chunks of the 262-entry
    table -> xT [16, 12800] directly in conv layout. The emb chunks are
    interleaved into the conv stream so they fill PE gaps while the DVE
    drains conv PSUM.
  - im2col by 6 shifted SBUF->SBUF DMA copies -> X7 [112+6, 12800+pad].
    Rows 112..117 are per-position-class indicator rows; the conv weight
    matrix carries -1e30 in those rows for (channel, position) pairs that
    are invalid, so max-over-time needs no masking pass.
  - All 7 convs as one packed [118, 2048] bf16 matmul into 2-bank PSUM
    tiles (16 tokens each); max-over-time as one strided DVE reduce per
    tile (the DVE is the only engine that can drain PSUM with a max; its
    ~226us of reduce work is the kernel's critical path).
  - Highway + projection in token-major form overlapped under the reduce
    stream: layer-0 token-half 0 runs while the conv's second half is
    still reducing. Weight slabs streamed from DRAM in bf16.
"""

import os
import numpy as np
import ml_dtypes

import concourse.bass as bass
import concourse.mybir as mybir
import concourse.tile as tile
from concourse.bass_utils import run_bass_kernel_spmd

F32 = mybir.dt.float32
BF16 = mybir.dt.bfloat16
NPBF16 = ml_dtypes.bfloat16

CNN_OPTIONS = [(1, 32), (2, 32), (3, 64), (4, 128), (5, 256), (6, 512), (7, 1024)]
EMB_DIM = 16
N_CHARS = 262
MAX_CHARS = 50
N_FILTERS = 2048
OUT_DIM = 512
N_HIGHWAY = 2
BATCH, SEQ = 4, 512
NCORES = 8
T_LOC = BATCH * SEQ // NCORES          # 256 tokens per core
COLS = T_LOC * MAX_CHARS               # 12800
COLS_PAD = COLS + 16                   # 12816
KMAX = 7
KROWS = EMB_DIM * KMAX                 # 112
KTOT = KROWS + 6                       # 118 (6 indicator rows for pos 44..49)
NCH = 512                              # xT build chunk width
NXCH = COLS // NCH                     # 25
TOKG = 8                               # tokens per conv matmul
# oc-chunk list: (chunk idx -> kernel size driving its valid-position count)
CHUNK_K = [1, 4, 5, 5, 6, 6, 6, 6, 7, 7, 7, 7, 7, 7, 7, 7]  # m=0 mixed (use 50 pos)
CHUNK_NP = [50 if k == 1 else (MAX_CHARS - k + 1) for k in CHUNK_K]
HWM = 32                               # 4096/128 output chunks per highway layer
KC = 16                                # 2048/128 contraction chunks


def _split_multi_waits(nc):
    """This walrus build encodes at most ONE sync-wait per instruction.
    Hoist extra waits onto dedicated NoOps ahead of the instruction."""
    ctr = [0]
    for f in nc.m.functions:
        for b in f.blocks:
            il = b.instructions
            if not any(
                i.sync_info is not None and len(i.sync_info.on_wait) > 1 for i in il
            ):
                continue
            new = []
            for ins in il:
                si = ins.sync_info
                if si is not None and len(si.on_wait) > 1:
                    waits = list(si.on_wait)
                    for w in waits[:-1]:
                        ctr[0] += 1
                        nop = mybir.InstNoOp(name=f"wsplit-{ctr[0]}", ins=[], outs=[])
                        nop.engine = ins.engine
                        nop.sync_info = mybir.SyncInfo(on_wait=[w], on_update=[])
                        new.append(nop)
                    ins.sync_info = mybir.SyncInfo(
                        on_wait=[waits[-1]], on_update=list(si.on_update)
                    )
                new.append(ins)
            b.instructions = new


def _build_program(split_waits=True):
    nc = bass.Bass(target_bir_lowering=False)

    oh_d = [
        nc.dram_tensor(f"oh{r}", [128, COLS_PAD], BF16, kind="ExternalInput")
        for r in range(3)
    ]
    embt_d = nc.dram_tensor("embt", [384, EMB_DIM], BF16, kind="ExternalInput")
    convw_d = nc.dram_tensor("convw", [KTOT, N_FILTERS], BF16, kind="ExternalInput")
    indic_d = nc.dram_tensor("indic", [6, COLS_PAD], BF16, kind="ExternalInput")
    cbias_d = nc.dram_tensor("cbias", [128, 16], F32, kind="ExternalInput")
    hw0_d = nc.dram_tensor("hw0", [KC, 4, 128, 1024], BF16, kind="ExternalInput")
    hw1_d = nc.dram_tensor("hw1", [KC, 4, 128, 1024], BF16, kind="ExternalInput")
    hb0_d = nc.dram_tensor("hb0", [1, 4096], BF16, kind="ExternalInput")
    hb1_d = nc.dram_tensor("hb1", [1, 4096], BF16, kind="ExternalInput")
    pw_d = nc.dram_tensor("pw", [KC, 128, 512], BF16, kind="ExternalInput")
    pb_d = nc.dram_tensor("pb", [1, 512], BF16, kind="ExternalInput")
    out_d = nc.dram_tensor("outT", [T_LOC, OUT_DIM], F32, kind="ExternalOutput")

    with tile.TileContext(nc) as tc:
        with (
            tc.tile_pool(name="const", bufs=1) as cpool,
            tc.tile_pool(name="elem", bufs=3) as epool,
            tc.tile_pool(name="outp", bufs=2) as outpool,
            tc.tile_pool(name="ps_big", bufs=2, space="PSUM") as ps_big,
            tc.tile_pool(name="ps_xt", bufs=2, space="PSUM") as ps_xt,
            tc.tile_pool(name="ps_hw", bufs=2, space="PSUM") as ps_hw,
        ):
            # one-hot encodings stream per 512-col chunk (3 tiles each);
            # the ring lets the SP run several chunks ahead of the PE
            ohpool_cm = tc.tile_pool(name="ohp", bufs=8)
            ohpool = ohpool_cm.__enter__()

            # ---- constants in ----
            embt_s = cpool.tile([128, 3 * EMB_DIM], BF16, tag="embt")
            for r in range(3):
                nc.sync.dma_start(
                    embt_s[:, 16 * r : 16 * r + 16], embt_d[128 * r : 128 * r + 128, :]
                )
            cbias_s = cpool.tile([128, 16], F32, tag="cbias")
            nc.sync.dma_start(cbias_s[:], cbias_d[:])
            # X7: rows 0-15 xT base, 16-111 shifted copies, 112-117 indicators
            X7 = cpool.tile([KTOT, COLS_PAD], BF16, tag="X7")
            convw_s = cpool.tile([KTOT, N_FILTERS], BF16, tag="convw")
            hb0_s = cpool.tile([1, 4096], BF16, tag="hb0")
            nc.sync.dma_start(hb0_s[:], hb0_d[:])
            hb1_s = cpool.tile([1, 4096], BF16, tag="hb1")
            nc.sync.dma_start(hb1_s[:], hb1_d[:])
            pb_s = cpool.tile([1, 512], BF16, tag="pb")
            nc.sync.dma_start(pb_s[:], pb_d[:])
            onesb_s = cpool.tile([1, 128], BF16, tag="onesb")
            nc.gpsimd.memset(onesb_s[:], 1.0)
            from concourse.masks import make_identity
            ident_s = cpool.tile([128, 128], BF16, tag="ident")
            make_identity(nc, ident_s[:])

            hts = [cpool.tile([128, T_LOC], BF16, tag=f"hT{m}", name=f"hT{m}") for m in range(KC)]
            h1ts = [cpool.tile([128, T_LOC], BF16, tag=f"h1T{m}", name=f"h1T{m}") for m in range(KC)]
            h2ts = [cpool.tile([128, T_LOC], BF16, tag=f"h2T{m}", name=f"h2T{m}") for m in range(KC)]
            htok = [cpool.tile([128, N_FILTERS], BF16, tag=f"htok{t}", name=f"htok{t}") for t in range(2)]
            h1tok = [cpool.tile([128, N_FILTERS], BF16, tag=f"h1tok{t}", name=f"h1tok{t}") for t in range(2)]

            # ---- helpers ----
            def transpose_block(src_ap, dst_ap, nm):
                tp = ps_xt.tile([128, 128], BF16, space="PSUM", tag="xt", name=nm)
                nc.tensor.transpose(tp[:], src_ap, ident_s[:])
                nc.scalar.copy(dst_ap, tp[:])

            # emb chunk: stream 3 one-hot chunk tiles in, 3 accumulated
            # matmuls -> X7 rows 0:16
            def emb_chunk(c):
                c0 = c * NCH
                ohs = []
                for r in range(3):
                    t = ohpool.tile([128, NCH], BF16, tag=f"oh{r}", name=f"oh{r}_{c}")
                    nc.sync.dma_start(t[:], oh_d[r][:, c0 : c0 + NCH])
                    ohs.append(t)
                px = ps_xt.tile([16, NCH], F32, space="PSUM", tag="xt", name=f"px{c}")
                for r in range(3):
                    nc.tensor.matmul(
                        px[:],
                        embt_s[:, 16 * r : 16 * r + 16],
                        ohs[r][:],
                        start=(r == 0),
                        stop=(r == 2),
                    )
                nc.scalar.copy(X7[0:16, c0 : c0 + NCH], px[:])

            def shifts(cl, cu):
                for j in range(1, KMAX):
                    nc.sync.dma_start(
                        X7[16 * j : 16 * j + 16, cl:cu], X7[0:16, cl + j : cu + j]
                    )

            # conv unit: 16 tokens x one 128-chan chunk, one strided reduce
            def conv_unit(nnp, m):
                npos = CHUNK_NP[m]
                ps = ps_big.tile(
                    [128, 1024], F32, space="PSUM", tag="big", name=f"cv{m}_{nnp}"
                )
                for h in range(2):
                    c0 = (2 * nnp + h) * TOKG * MAX_CHARS
                    rhs = (
                        X7[0:KTOT, c0 : c0 + TOKG * MAX_CHARS]
                        .rearrange("p (t c) -> p t c", c=MAX_CHARS)[:, :, 0:npos]
                    )
                    nc.tensor.matmul(
                        ps[:, 512 * h : 512 * h + TOKG * npos],
                        convw_s[:, 128 * m : 128 * m + 128],
                        rhs,
                        start=True,
                        stop=True,
                    )
                ps4 = (
                    ps[:]
                    .rearrange("p (h x) -> p h x", h=2)[:, :, 0 : TOKG * npos]
                    .rearrange("p h (t c) -> p h t c", c=npos)
                )
                dst = hts[m][:, 16 * nnp : 16 * nnp + 16].rearrange(
                    "p (h t) -> p h t", h=2
                )
                nc.vector.reduce_max(dst, ps4, axis=mybir.AxisListType.X)

            def finish_half(half):
                for m in range(16):
                    hsl = hts[m][:, 128 * half : 128 * half + 128]
                    nc.scalar.activation(
                        hsl, hsl, mybir.ActivationFunctionType.Relu,
                        bias=cbias_s[:, m : m + 1], scale=1.0,
                    )
                for c in range(KC):
                    transpose_block(
                        hts[c][:, 128 * half : 128 * half + 128],
                        htok[half][:, 128 * c : 128 * c + 128],
                        f"tpc{half}_{c}",
                    )

            # one DMA per (layer, g, k-half): 8 k-slabs land in a single
            # [128, 8*1024] group tile (8x fewer SP issues and sem hops)
            def hw_grp_dma(gpool, lyr, g, mt):
                w_d = hw0_d if lyr == 0 else hw1_d
                grps = []
                for hk in range(2):
                    grp = gpool.tile(
                        [128, 8 * 1024], BF16, tag="wgrp",
                        name=f"wg{lyr}{g}{mt}{hk}",
                    )
                    nc.sync.dma_start(
                        grp[:].rearrange("p (k x) -> p k x", k=8),
                        w_d[8 * hk : 8 * hk + 8, g].rearrange("k p x -> p k x"),
                    )
                    grps.append(grp)
                return grps

            def hw_chain(lyr, g, bb, mt, grp):
                src_ts = hts if lyr == 0 else h1ts
                h_in = htok if lyr == 0 else h1tok
                hb_s = hb0_s if lyr == 0 else hb1_s
                b = 2 * g + bb
                ps = ps_hw.tile(
                    [128, 512], F32, space="PSUM", tag="hwp",
                    name=f"hwp{lyr}{b}{mt}",
                )
                for k in range(KC):
                    gh = grp[k // 8]
                    o = 1024 * (k % 8) + 512 * bb
                    nc.tensor.matmul(
                        ps[:],
                        src_ts[k][:, 128 * mt : 128 * mt + 128],
                        gh[:, o : o + 512],
                        start=(k == 0), stop=False,
                    )
                nc.tensor.matmul(
                    ps[:], onesb_s[0:1, :],
                    hb_s[0:1, 512 * b : 512 * b + 512],
                    start=False, stop=True,
                )
                r_ = epool.tile([128, 256], BF16, tag="relu", name=f"r{lyr}{b}{mt}")
                nc.scalar.activation(
                    r_[:], ps[:, 0:256], mybir.ActivationFunctionType.Relu
                )
                g_ = epool.tile([128, 256], BF16, tag="gate", name=f"g{lyr}{b}{mt}")
                nc.scalar.activation(
                    g_[:], ps[:, 256:512], mybir.ActivationFunctionType.Sigmoid
                )
                hsl = h_in[mt][:, 256 * b : 256 * b + 256]
                t1 = epool.tile([128, 256], BF16, tag="t1", name=f"t1_{lyr}{b}{mt}")
                nc.vector.tensor_tensor(
                    out=t1[:], in0=hsl, in1=r_[:], op=mybir.AluOpType.subtract
                )
                t2 = epool.tile([128, 256], BF16, tag="t2", name=f"t2_{lyr}{b}{mt}")
                nc.vector.tensor_tensor(
                    out=t2[:], in0=g_[:], in1=t1[:], op=mybir.AluOpType.mult
                )
                if lyr == 0:
                    dst_sl = h1tok[mt][:, 256 * b : 256 * b + 256]
                    nc.vector.tensor_tensor(
                        out=dst_sl, in0=t2[:], in1=r_[:], op=mybir.AluOpType.add
                    )
                    for c in (2 * b, 2 * b + 1):
                        transpose_block(
                            h1tok[mt][:, 128 * c : 128 * c + 128],
                            h1ts[c][:, 128 * mt : 128 * mt + 128],
                            f"tp1_{b}{mt}{c}",
                        )
                else:
                    h2sl = epool.tile(
                        [128, 256], BF16, tag="h2sl", name=f"h2sl{b}{mt}"
                    )
                    nc.vector.tensor_tensor(
                        out=h2sl[:], in0=t2[:], in1=r_[:], op=mybir.AluOpType.add
                    )
                    for ci, c in enumerate((2 * b, 2 * b + 1)):
                        transpose_block(
                            h2sl[:, 128 * ci : 128 * ci + 128],
                            h2ts[c][:, 128 * mt : 128 * mt + 128],
                            f"tp2_{b}{mt}{c}",
                        )

            # ---- pipelined schedule ----
            # emb chunks 0..8 unlock shift block 0 (cols 0..3578, reads to
            # 3583); conv token-half 0 (pairs 0..7) interleaves the remaining
            # emb chunks so the PE fills its reduce-paced gaps. Shift blocks
            # fire as soon as their emb chunks are done. The big const DMAs
            # (conv weights, indicators) are issued after the first few
            # one-hot chunks so the emb pipeline starts immediately.
            for c in range(3):
                emb_chunk(c)
            nc.sync.dma_start(convw_s[:], convw_d[:])
            nc.sync.dma_start(X7[112:118, :], indic_d[:])
            nc.gpsimd.memset(X7[0:16, COLS:COLS_PAD], 0.0)
            for c in range(3, 9):
                emb_chunk(c)
            shifts(0, 3578)
            next_c = [9]
            SH = {13: (3578, 6650), 19: (6650, 9722), 25: (9722, COLS + 6)}

            def pump_emb():
                if next_c[0] >= NXCH:
                    return
                c = next_c[0]
                emb_chunk(c)
                next_c[0] += 1
                if c + 1 in SH:
                    shifts(*SH[c + 1])

            u = 0
            for nnp in range(8):
                for m in range(16):
                    conv_unit(nnp, m)
                    u += 1
                    if u % 8 == 0:
                        pump_emb()
            while next_c[0] < NXCH:
                pump_emb()
            finish_half(0)

            with tc.tile_pool(name="wgrp", bufs=5) as gpool:
                # conv token-half 1 hosts the mt=0 highway work (both layers)
                # in its PE gaps; the mt=1 passes + projection trail it, with
                # their group DMAs streaming early thanks to the 3-deep ring.
                chains = [(0, g, bb) for g in range(4) for bb in range(2)] + [
                    (1, g, bb) for g in range(4) for bb in range(2)
                ]
                chains.append(("proj", 0, 0))
                grp_cache = {}
                pgrps = []

                def proj_mt(mt):
                    if not pgrps:
                        for hk in range(2):
                            pg = gpool.tile(
                                [128, 8 * 512], BF16, tag="wgrp", name=f"pgrp{hk}"
                            )
                            nc.sync.dma_start(
                                pg[:].rearrange("p (k x) -> p k x", k=8),
                                pw_d[8 * hk : 8 * hk + 8].rearrange("k p x -> p k x"),
                            )
                            pgrps.append(pg)
                    ps = ps_hw.tile(
                        [128, 512], F32, space="PSUM", tag="hwp", name=f"pj{mt}"
                    )
                    for k in range(KC):
                        nc.tensor.matmul(
                            ps[:],
                            h2ts[k][:, 128 * mt : 128 * mt + 128],
                            pgrps[k // 8][:, 512 * (k % 8) : 512 * (k % 8) + 512],
                            start=(k == 0), stop=False,
                        )
                    nc.tensor.matmul(
                        ps[:], onesb_s[0:1, :], pb_s[0:1, :], start=False, stop=True
                    )
                    oc = outpool.tile([128, 512], F32, tag="out", name=f"oc{mt}")
                    nc.scalar.copy(oc[:], ps[:])
                    nc.sync.dma_start(out_d[128 * mt : 128 * mt + 128, :], oc[:])

                def emit_chain(lyr, g, bb, mt):
                    if lyr == "proj":
                        proj_mt(mt)
                        return
                    key = (lyr, g, mt)
                    if key not in grp_cache:
                        grp_cache[key] = hw_grp_dma(gpool, lyr, g, mt)
                    hw_chain(lyr, g, bb, mt, grp_cache[key])

                u = 0
                for nnp in range(8, 16):
                    for m in range(16):
                        conv_unit(nnp, m)
                        u += 1
                        if u % 8 == 0 and chains:
                            emit_chain(*chains.pop(0), 0)
                for lyr, g, bb in chains:
                    emit_chain(lyr, g, bb, 0)
                finish_half(1)
                for g in range(4):
                    for bb in range(2):
                        emit_chain(0, g, bb, 1)
                for g in range(4):
                    for bb in range(2):
                        emit_chain(1, g, bb, 1)
                proj_mt(1)
            ohpool_cm.__exit__(None, None, None)

    if split_waits:
        _split_multi_waits(nc)
    return nc


def _prep_weights(inputs):
    conv_ws = [np.asarray(inputs[f"conv_w{i}"], np.float32) for i in range(7)]
    conv_bs = [np.asarray(inputs[f"conv_b{i}"], np.float32) for i in range(7)]

    W7 = np.zeros((KTOT, N_FILTERS), np.float32)
    o0 = 0
    for (ksz, oc), w in zip(CNN_OPTIONS, conv_ws):
        for j in range(ksz):
            W7[16 * j : 16 * j + 16, o0 : o0 + oc] = w[:, :, j].T
        # indicator-mask rows: position p = 44+i invalid iff p > 50-ksz
        for i in range(6):
            if (44 + i) > (MAX_CHARS - ksz):
                W7[KROWS + i, o0 : o0 + oc] = -1e30
        o0 += oc

    b_all = np.concatenate(conv_bs)
    cbias = b_all.reshape(16, 128).T.astype(np.float32)

    indic = np.zeros((6, COLS_PAD), np.float32)
    for i in range(6):
        indic[i, (44 + i) : COLS : MAX_CHARS] = 1.0

    emb = np.asarray(inputs["emb"], np.float32)
    embt = np.zeros((384, EMB_DIM), np.float32)
    embt[:N_CHARS] = emb

    # token-form highway: out = h @ w'.T computed as hT-chunks.T @ w'.T-slabs.
    # Channel-interleave so slab b holds [256 nonlin | 256 gate] of block b.
    perm = np.concatenate(
        [
            np.concatenate([np.arange(256 * b, 256 * b + 256),
                            2048 + np.arange(256 * b, 256 * b + 256)])
            for b in range(8)
        ]
    )

    def hw_slabs(w, bvec):
        wp = np.asarray(w, np.float32)[perm]       # [4096, 2048] permuted rows
        bp = np.asarray(bvec, np.float32)[perm]
        wt = wp.T                                   # [2048, 4096]
        arr = wt.reshape(KC, 128, 8, 512).transpose(0, 2, 1, 3)  # [16, 8, 128, 512]
        arr = arr.reshape(KC, 4, 2, 128, 512).transpose(0, 1, 3, 2, 4).reshape(KC, 4, 128, 1024)
        return np.ascontiguousarray(arr).astype(NPBF16), bp[None, :].astype(NPBF16)

    hw0, hb0 = hw_slabs(inputs["hw_w0"], inputs["hw_b0"])
    hw1, hb1 = hw_slabs(inputs["hw_w1"], inputs["hw_b1"])
    pwt = np.asarray(inputs["proj_w"], np.float32).T  # [2048, 512]
    pw = np.ascontiguousarray(pwt.reshape(KC, 128, 512)).astype(NPBF16)
    pb = np.asarray(inputs["proj_b"], np.float32)[None, :].astype(NPBF16)

    return {
        "embt": embt.astype(NPBF16),
        "convw": W7.astype(NPBF16),
        "indic": indic.astype(NPBF16),
        "cbias": cbias,
        "hw0": hw0.astype(NPBF16),
        "hw1": hw1.astype(NPBF16),
        "hb0": hb0,
        "hb1": hb1,
        "pw": pw.astype(NPBF16),
        "pb": pb,
    }


_NC_CACHE = []
LAST_RESULT = {}


def kernel(**inputs) -> np.ndarray:
    if not _NC_CACHE:
        _NC_CACHE.append(_build_program())
    nc = _NC_CACHE[0]

    shared = _prep_weights(inputs)
    ids = np.asarray(inputs["batch_ids"]).astype(np.int64).reshape(-1, MAX_CHARS)
    rng384 = np.arange(384, dtype=np.int64)
    in_maps = []
    for core in range(NCORES):
        flat = ids[core * T_LOC : (core + 1) * T_LOC].reshape(-1)
        oh = np.zeros((384, COLS_PAD), NPBF16)
        oh[:, :COLS] = (flat[None, :] == rng384[:, None]).astype(NPBF16)
        cm = {f"oh{r}": np.ascontiguousarray(oh[128 * r : 128 * r + 128]) for r in range(3)}
        in_maps.append({**cm, **shared})

    trace = bool(int(os.environ.get("KERNEL_TRACE", "0")))
    res = run_bass_kernel_spmd(
        nc, in_maps, core_ids=list(range(NCORES)), trace=trace
    )
    LAST_RESULT["exec_time_ns"] = res.exec_time_ns
    LAST_RESULT["trace"] = res.instructions_and_trace

    parts = [res.results[c]["outT"] for c in range(NCORES)]  # each [256, 512]
    out = np.concatenate(parts, axis=0).reshape(BATCH, SEQ, OUT_DIM)
    return np.ascontiguousarray(out.astype(np.float32))


# revision 41
# speedup vs baseline: 1.0714x; 1.0185x over previous
"""CharCNN embedder (ELMo-style) Trainium2 Bass kernel, v3.

Strategy (pure data parallel over 8 cores, 256 tokens each):
  - Embedding lookup as one-hot matmul: one-hot encodings of the char ids
    arrive pre-built from the host (3 x [128, COLS_PAD] bf16, exact 0/1);
    the device runs embT.T @ onehot over 3 row-chunks of the 262-entry
    table -> xT [16, 12800] directly in conv layout. The emb chunks are
    interleaved into the conv stream so they fill PE gaps while the DVE
    drains conv PSUM.
  - im2col by 6 shifted SBUF->SBUF DMA copies -> X7 [112+6, 12800+pad].
    Rows 112..117 are per-position-class indicator rows; the conv weight
    matrix carries -1e30 in those rows for (channel, position) pairs that
    are invalid, so max-over-time needs no masking pass.
  - All 7 convs as one packed [118, 2048] bf16 matmul into 2-bank PSUM
    tiles (16 tokens each); max-over-time as one strided DVE reduce per
    tile (the DVE is the only engine that can drain PSUM with a max; its
    ~226us of reduce work is the kernel's critical path).
  - Highway + projection in token-major form overlapped under the reduce
    stream: layer-0 token-half 0 runs while the conv's second half is
    still reducing. Weight slabs streamed from DRAM in bf16.
"""

import os
import numpy as np
import ml_dtypes

import concourse.bass as bass
import concourse.mybir as mybir
import concourse.tile as tile
from concourse.bass_utils import run_bass_kernel_spmd

F32 = mybir.dt.float32
BF16 = mybir.dt.bfloat16
NPBF16 = ml_dtypes.bfloat16

CNN_OPTIONS = [(1, 32), (2, 32), (3, 64), (4, 128), (5, 256), (6, 512), (7, 1024)]
EMB_DIM = 16
N_CHARS = 262
MAX_CHARS = 50
N_FILTERS = 2048
OUT_DIM = 512
N_HIGHWAY = 2
BATCH, SEQ = 4, 512
NCORES = 8
T_LOC = BATCH * SEQ // NCORES          # 256 tokens per core
COLS = T_LOC * MAX_CHARS               # 12800
COLS_PAD = COLS + 16                   # 12816
KMAX = 7
KROWS = EMB_DIM * KMAX                 # 112
KTOT = KROWS + 6                       # 118 (6 indicator rows for pos 44..49)
NCH = 512                              # xT build chunk width
NXCH = COLS // NCH                     # 25
TOKG = 8                               # tokens per conv matmul
# oc-chunk list: (chunk idx -> kernel size driving its valid-position count)
CHUNK_K = [1, 4, 5, 5, 6, 6, 6, 6, 7, 7, 7, 7, 7, 7, 7, 7]  # m=0 mixed (use 50 pos)
CHUNK_NP = [50 if k == 1 else (MAX_CHARS - k + 1) for k in CHUNK_K]
HWM = 32                               # 4096/128 output chunks per highway layer
KC = 16                                # 2048/128 contraction chunks


def _split_multi_waits(nc):
    """This walrus build encodes at most ONE sync-wait per instruction.
    Hoist extra waits onto dedicated NoOps ahead of the instruction."""
    ctr = [0]
    for f in nc.m.functions:
        for b in f.blocks:
            il = b.instructions
            if not any(
                i.sync_info is not None and len(i.sync_info.on_wait) > 1 for i in il
            ):
                continue
            new = []
            for ins in il:
                si = ins.sync_info
                if si is not None and len(si.on_wait) > 1:
                    waits = list(si.on_wait)
                    for w in waits[:-1]:
                        ctr[0] += 1
                        nop = mybir.InstNoOp(name=f"wsplit-{ctr[0]}", ins=[], outs=[])
                        nop.engine = ins.engine
                        nop.sync_info = mybir.SyncInfo(on_wait=[w], on_update=[])
                        new.append(nop)
                    ins.sync_info = mybir.SyncInfo(
                        on_wait=[waits[-1]], on_update=list(si.on_update)
                    )
                new.append(ins)
            b.instructions = new


def _build_program(split_waits=True):
    nc = bass.Bass(target_bir_lowering=False)

    oh_d = [
        nc.dram_tensor(f"oh{r}", [128, COLS_PAD], BF16, kind="ExternalInput")
        for r in range(3)
    ]
    embt_d = nc.dram_tensor("embt", [384, EMB_DIM], BF16, kind="ExternalInput")
    convw_d = nc.dram_tensor("convw", [KTOT, N_FILTERS], BF16, kind="ExternalInput")
    indic_d = nc.dram_tensor("indic", [6, COLS_PAD], BF16, kind="ExternalInput")
    cbias_d = nc.dram_tensor("cbias", [128, 16], F32, kind="ExternalInput")
    hw0_d = nc.dram_tensor("hw0", [KC, 4, 128, 1024], BF16, kind="ExternalInput")
    hw1_d = nc.dram_tensor("hw1", [KC, 4, 128, 1024], BF16, kind="ExternalInput")
    hb0_d = nc.dram_tensor("hb0", [1, 4096], BF16, kind="ExternalInput")
    hb1_d = nc.dram_tensor("hb1", [1, 4096], BF16, kind="ExternalInput")
    pw_d = nc.dram_tensor("pw", [KC, 128, 512], BF16, kind="ExternalInput")
    pb_d = nc.dram_tensor("pb", [1, 512], BF16, kind="ExternalInput")
    out_d = nc.dram_tensor("outT", [T_LOC, OUT_DIM], F32, kind="ExternalOutput")

    with tile.TileContext(nc) as tc:
        with (
            tc.tile_pool(name="const", bufs=1) as cpool,
            tc.tile_pool(name="elem", bufs=2) as epool,
            tc.tile_pool(name="outp", bufs=2) as outpool,
            tc.tile_pool(name="ps_big", bufs=2, space="PSUM") as ps_big,
            tc.tile_pool(name="ps_xt", bufs=2, space="PSUM") as ps_xt,
            tc.tile_pool(name="ps_hw", bufs=2, space="PSUM") as ps_hw,
        ):
            # one-hot encodings stream per 512-col chunk (3 tiles each);
            # the ring lets the SP run several chunks ahead of the PE
            ohpool_cm = tc.tile_pool(name="ohp", bufs=5)
            ohpool = ohpool_cm.__enter__()

            # ---- constants in ----
            embt_s = cpool.tile([128, 3 * EMB_DIM], BF16, tag="embt")
            for r in range(3):
                nc.sync.dma_start(
                    embt_s[:, 16 * r : 16 * r + 16], embt_d[128 * r : 128 * r + 128, :]
                )
            cbias_s = cpool.tile([128, 16], F32, tag="cbias")
            nc.sync.dma_start(cbias_s[:], cbias_d[:])
            # X7: rows 0-15 xT base, 16-111 shifted copies, 112-117 indicators
            X7 = cpool.tile([KTOT, COLS_PAD], BF16, tag="X7")
            convw_s = cpool.tile([KTOT, N_FILTERS], BF16, tag="convw")
            hb0_s = cpool.tile([1, 4096], BF16, tag="hb0")
            nc.sync.dma_start(hb0_s[:], hb0_d[:])
            hb1_s = cpool.tile([1, 4096], BF16, tag="hb1")
            nc.sync.dma_start(hb1_s[:], hb1_d[:])
            pb_s = cpool.tile([1, 512], BF16, tag="pb")
            nc.sync.dma_start(pb_s[:], pb_d[:])
            onesb_s = cpool.tile([1, 128], BF16, tag="onesb")
            nc.gpsimd.memset(onesb_s[:], 1.0)
            from concourse.masks import make_identity
            ident_s = cpool.tile([128, 128], BF16, tag="ident")
            make_identity(nc, ident_s[:])

            hts = [cpool.tile([128, T_LOC], BF16, tag=f"hT{m}", name=f"hT{m}") for m in range(KC)]
            h1ts = [cpool.tile([128, T_LOC], BF16, tag=f"h1T{m}", name=f"h1T{m}") for m in range(KC)]
            h2ts = [cpool.tile([128, T_LOC], BF16, tag=f"h2T{m}", name=f"h2T{m}") for m in range(KC)]
            htok = [cpool.tile([128, N_FILTERS], BF16, tag=f"htok{t}", name=f"htok{t}") for t in range(2)]
            h1tok = [cpool.tile([128, N_FILTERS], BF16, tag=f"h1tok{t}", name=f"h1tok{t}") for t in range(2)]

            # ---- helpers ----
            def transpose_block(src_ap, dst_ap, nm):
                tp = ps_xt.tile([128, 128], BF16, space="PSUM", tag="xt", name=nm)
                nc.tensor.transpose(tp[:], src_ap, ident_s[:])
                nc.scalar.copy(dst_ap, tp[:])

            # emb chunk: stream 3 one-hot chunk tiles in, 3 accumulated
            # matmuls -> X7 rows 0:16
            def emb_chunk(c):
                c0 = c * NCH
                ohs = []
                for r in range(3):
                    t = ohpool.tile([128, NCH], BF16, tag=f"oh{r}", name=f"oh{r}_{c}")
                    nc.sync.dma_start(t[:], oh_d[r][:, c0 : c0 + NCH])
                    ohs.append(t)
                px = ps_xt.tile([16, NCH], F32, space="PSUM", tag="xt", name=f"px{c}")
                for r in range(3):
                    nc.tensor.matmul(
                        px[:],
                        embt_s[:, 16 * r : 16 * r + 16],
                        ohs[r][:],
                        start=(r == 0),
                        stop=(r == 2),
                    )
                nc.scalar.copy(X7[0:16, c0 : c0 + NCH], px[:])

            def shifts(cl, cu):
                for j in range(1, KMAX):
                    nc.sync.dma_start(
                        X7[16 * j : 16 * j + 16, cl:cu], X7[0:16, cl + j : cu + j]
                    )

            # conv unit: 16 tokens x one 128-chan chunk, one strided reduce
            def conv_unit(nnp, m):
                npos = CHUNK_NP[m]
                ps = ps_big.tile(
                    [128, 1024], F32, space="PSUM", tag="big", name=f"cv{m}_{nnp}"
                )
                for h in range(2):
                    c0 = (2 * nnp + h) * TOKG * MAX_CHARS
                    rhs = (
                        X7[0:KTOT, c0 : c0 + TOKG * MAX_CHARS]
                        .rearrange("p (t c) -> p t c", c=MAX_CHARS)[:, :, 0:npos]
                    )
                    nc.tensor.matmul(
                        ps[:, 512 * h : 512 * h + TOKG * npos],
                        convw_s[:, 128 * m : 128 * m + 128],
                        rhs,
                        start=True,
                        stop=True,
                    )
                ps4 = (
                    ps[:]
                    .rearrange("p (h x) -> p h x", h=2)[:, :, 0 : TOKG * npos]
                    .rearrange("p h (t c) -> p h t c", c=npos)
                )
                dst = hts[m][:, 16 * nnp : 16 * nnp + 16].rearrange(
                    "p (h t) -> p h t", h=2
                )
                nc.vector.reduce_max(dst, ps4, axis=mybir.AxisListType.X)

            def finish_half(half):
                for m in range(16):
                    hsl = hts[m][:, 128 * half : 128 * half + 128]
                    nc.scalar.activation(
                        hsl, hsl, mybir.ActivationFunctionType.Relu,
                        bias=cbias_s[:, m : m + 1], scale=1.0,
                    )
                for c in range(KC):
                    transpose_block(
                        hts[c][:, 128 * half : 128 * half + 128],
                        htok[half][:, 128 * c : 128 * c + 128],
                        f"tpc{half}_{c}",
                    )

            # one DMA per (layer, g, k-half): 8 k-slabs land in a single
            # [128, 8*1024] group tile (8x fewer SP issues and sem hops)
            def hw_grp_dma(gpool, lyr, g, mt):
                w_d = hw0_d if lyr == 0 else hw1_d
                grps = []
                for hk in range(2):
                    grp = gpool.tile(
                        [128, 8 * 1024], BF16, tag="wgrp",
                        name=f"wg{lyr}{g}{mt}{hk}",
                    )
                    nc.sync.dma_start(
                        grp[:].rearrange("p (k x) -> p k x", k=8),
                        w_d[8 * hk : 8 * hk + 8, g].rearrange("k p x -> p k x"),
                    )
                    grps.append(grp)
                return grps

            def hw_chain(lyr, g, bb, mt, grp):
                src_ts = hts if lyr == 0 else h1ts
                h_in = htok if lyr == 0 else h1tok
                hb_s = hb0_s if lyr == 0 else hb1_s
                b = 2 * g + bb
                ps = ps_hw.tile(
                    [128, 512], F32, space="PSUM", tag="hwp",
                    name=f"hwp{lyr}{b}{mt}",
                )
                for k in range(KC):
                    gh = grp[k // 8]
                    o = 1024 * (k % 8) + 512 * bb
                    nc.tensor.matmul(
                        ps[:],
                        src_ts[k][:, 128 * mt : 128 * mt + 128],
                        gh[:, o : o + 512],
                        start=(k == 0), stop=False,
                    )
                nc.tensor.matmul(
                    ps[:], onesb_s[0:1, :],
                    hb_s[0:1, 512 * b : 512 * b + 512],
                    start=False, stop=True,
                )
                r_ = epool.tile([128, 256], BF16, tag="relu", name=f"r{lyr}{b}{mt}")
                nc.scalar.activation(
                    r_[:], ps[:, 0:256], mybir.ActivationFunctionType.Relu
                )
                g_ = epool.tile([128, 256], BF16, tag="gate", name=f"g{lyr}{b}{mt}")
                nc.scalar.activation(
                    g_[:], ps[:, 256:512], mybir.ActivationFunctionType.Sigmoid
                )
                hsl = h_in[mt][:, 256 * b : 256 * b + 256]
                t1 = epool.tile([128, 256], BF16, tag="t1", name=f"t1_{lyr}{b}{mt}")
                nc.vector.tensor_tensor(
                    out=t1[:], in0=hsl, in1=r_[:], op=mybir.AluOpType.subtract
                )
                t2 = epool.tile([128, 256], BF16, tag="t2", name=f"t2_{lyr}{b}{mt}")
                nc.vector.tensor_tensor(
                    out=t2[:], in0=g_[:], in1=t1[:], op=mybir.AluOpType.mult
                )
                if lyr == 0:
                    dst_sl = h1tok[mt][:, 256 * b : 256 * b + 256]
                    nc.vector.tensor_tensor(
                        out=dst_sl, in0=t2[:], in1=r_[:], op=mybir.AluOpType.add
                    )
                    for c in (2 * b, 2 * b + 1):
                        transpose_block(
                            h1tok[mt][:, 128 * c : 128 * c + 128],
                            h1ts[c][:, 128 * mt : 128 * mt + 128],
                            f"tp1_{b}{mt}{c}",
                        )
                else:
                    h2sl = epool.tile(
                        [128, 256], BF16, tag="h2sl", name=f"h2sl{b}{mt}"
                    )
                    nc.vector.tensor_tensor(
                        out=h2sl[:], in0=t2[:], in1=r_[:], op=mybir.AluOpType.add
                    )
                    for ci, c in enumerate((2 * b, 2 * b + 1)):
                        transpose_block(
                            h2sl[:, 128 * ci : 128 * ci + 128],
                            h2ts[c][:, 128 * mt : 128 * mt + 128],
                            f"tp2_{b}{mt}{c}",
                        )

            # ---- pipelined schedule ----
            # emb chunks 0..8 unlock shift block 0 (cols 0..3578, reads to
            # 3583); conv token-half 0 (pairs 0..7) interleaves the remaining
            # emb chunks so the PE fills its reduce-paced gaps. Shift blocks
            # fire as soon as their emb chunks are done. The big const DMAs
            # (conv weights, indicators) are issued after the first few
            # one-hot chunks so the emb pipeline starts immediately.
            for c in range(3):
                emb_chunk(c)
            nc.sync.dma_start(convw_s[:], convw_d[:])
            nc.sync.dma_start(X7[112:118, :], indic_d[:])
            nc.gpsimd.memset(X7[0:16, COLS:COLS_PAD], 0.0)
            emb_chunk(3)
            shifts(0, 1610)
            for c in range(4, 9):
                emb_chunk(c)
            shifts(1610, 3578)
            next_c = [9]
            SH = {13: (3578, 6650), 19: (6650, 9722), 25: (9722, COLS + 6)}

            def pump_emb():
                if next_c[0] >= NXCH:
                    return
                c = next_c[0]
                emb_chunk(c)
                next_c[0] += 1
                if c + 1 in SH:
                    shifts(*SH[c + 1])

            u = 0
            for nnp in range(8):
                for m in range(16):
                    conv_unit(nnp, m)
                    u += 1
                    if u % 8 == 0:
                        pump_emb()
            while next_c[0] < NXCH:
                pump_emb()
            finish_half(0)

            with tc.tile_pool(name="wgrp", bufs=6) as gpool:
                # conv token-half 1 hosts the mt=0 highway work (both layers)
                # in its PE gaps; the mt=1 passes + projection trail it, with
                # their group DMAs streaming early thanks to the 3-deep ring.
                chains = [(0, g, bb) for g in range(4) for bb in range(2)] + [
                    (1, g, bb) for g in range(4) for bb in range(2)
                ]
                chains.append(("proj", 0, 0))
                grp_cache = {}
                pgrps = []

                def proj_mt(mt):
                    if not pgrps:
                        for hk in range(2):
                            pg = gpool.tile(
                                [128, 8 * 512], BF16, tag="wgrp", name=f"pgrp{hk}"
                            )
                            nc.sync.dma_start(
                                pg[:].rearrange("p (k x) -> p k x", k=8),
                                pw_d[8 * hk : 8 * hk + 8].rearrange("k p x -> p k x"),
                            )
                            pgrps.append(pg)
                    ps = ps_hw.tile(
                        [128, 512], F32, space="PSUM", tag="hwp", name=f"pj{mt}"
                    )
                    for k in range(KC):
                        nc.tensor.matmul(
                            ps[:],
                            h2ts[k][:, 128 * mt : 128 * mt + 128],
                            pgrps[k // 8][:, 512 * (k % 8) : 512 * (k % 8) + 512],
                            start=(k == 0), stop=False,
                        )
                    nc.tensor.matmul(
                        ps[:], onesb_s[0:1, :], pb_s[0:1, :], start=False, stop=True
                    )
                    oc = outpool.tile([128, 512], F32, tag="out", name=f"oc{mt}")
                    nc.scalar.copy(oc[:], ps[:])
                    nc.sync.dma_start(out_d[128 * mt : 128 * mt + 128, :], oc[:])

                def emit_chain(lyr, g, bb, mt):
                    if lyr == "proj":
                        proj_mt(mt)
                        return
                    key = (lyr, g, mt)
                    if key not in grp_cache:
                        grp_cache[key] = hw_grp_dma(gpool, lyr, g, mt)
                    hw_chain(lyr, g, bb, mt, grp_cache[key])

                u = 0
                for nnp in range(8, 16):
                    for m in range(16):
                        conv_unit(nnp, m)
                        u += 1
                        if u % 8 == 0 and chains:
                            emit_chain(*chains.pop(0), 0)
                for lyr, g, bb in chains:
                    emit_chain(lyr, g, bb, 0)
                finish_half(1)
                for g in range(4):
                    for bb in range(2):
                        emit_chain(0, g, bb, 1)
                for g in range(4):
                    for bb in range(2):
                        emit_chain(1, g, bb, 1)
                proj_mt(1)
            ohpool_cm.__exit__(None, None, None)

    if split_waits:
        _split_multi_waits(nc)
    return nc


def _prep_weights(inputs):
    conv_ws = [np.asarray(inputs[f"conv_w{i}"], np.float32) for i in range(7)]
    conv_bs = [np.asarray(inputs[f"conv_b{i}"], np.float32) for i in range(7)]

    W7 = np.zeros((KTOT, N_FILTERS), np.float32)
    o0 = 0
    for (ksz, oc), w in zip(CNN_OPTIONS, conv_ws):
        for j in range(ksz):
            W7[16 * j : 16 * j + 16, o0 : o0 + oc] = w[:, :, j].T
        # indicator-mask rows: position p = 44+i invalid iff p > 50-ksz
        for i in range(6):
            if (44 + i) > (MAX_CHARS - ksz):
                W7[KROWS + i, o0 : o0 + oc] = -1e30
        o0 += oc

    b_all = np.concatenate(conv_bs)
    cbias = b_all.reshape(16, 128).T.astype(np.float32)

    indic = np.zeros((6, COLS_PAD), np.float32)
    for i in range(6):
        indic[i, (44 + i) : COLS : MAX_CHARS] = 1.0

    emb = np.asarray(inputs["emb"], np.float32)
    embt = np.zeros((384, EMB_DIM), np.float32)
    embt[:N_CHARS] = emb

    # token-form highway: out = h @ w'.T computed as hT-chunks.T @ w'.T-slabs.
    # Channel-interleave so slab b holds [256 nonlin | 256 gate] of block b.
    perm = np.concatenate(
        [
            np.concatenate([np.arange(256 * b, 256 * b + 256),
                            2048 + np.arange(256 * b, 256 * b + 256)])
            for b in range(8)
        ]
    )

    def hw_slabs(w, bvec):
        wp = np.asarray(w, np.float32)[perm]       # [4096, 2048] permuted rows
        bp = np.asarray(bvec, np.float32)[perm]
        wt = wp.T                                   # [2048, 4096]
        arr = wt.reshape(KC, 128, 8, 512).transpose(0, 2, 1, 3)  # [16, 8, 128, 512]
        arr = arr.reshape(KC, 4, 2, 128, 512).transpose(0, 1, 3, 2, 4).reshape(KC, 4, 128, 1024)
        return np.ascontiguousarray(arr).astype(NPBF16), bp[None, :].astype(NPBF16)

    hw0, hb0 = hw_slabs(inputs["hw_w0"], inputs["hw_b0"])
    hw1, hb1 = hw_slabs(inputs["hw_w1"], inputs["hw_b1"])
    pwt = np.asarray(inputs["proj_w"], np.float32).T  # [2048, 512]
    pw = np.ascontiguousarray(pwt.reshape(KC, 128, 512)).astype(NPBF16)
    pb = np.asarray(inputs["proj_b"], np.float32)[None, :].astype(NPBF16)

    return {
        "embt": embt.astype(NPBF16),
        "convw": W7.astype(NPBF16),
        "indic": indic.astype(NPBF16),
        "cbias": cbias,
        "hw0": hw0.astype(NPBF16),
        "hw1": hw1.astype(NPBF16),
        "hb0": hb0,
        "hb1": hb1,
        "pw": pw.astype(NPBF16),
        "pb": pb,
    }


_NC_CACHE = []
LAST_RESULT = {}


def kernel(**inputs) -> np.ndarray:
    if not _NC_CACHE:
        _NC_CACHE.append(_build_program())
    nc = _NC_CACHE[0]

    shared = _prep_weights(inputs)
    ids = np.asarray(inputs["batch_ids"]).astype(np.int64).reshape(-1, MAX_CHARS)
    rng384 = np.arange(384, dtype=np.int64)
    in_maps = []
    for core in range(NCORES):
        flat = ids[core * T_LOC : (core + 1) * T_LOC].reshape(-1)
        oh = np.zeros((384, COLS_PAD), NPBF16)
        oh[:, :COLS] = (flat[None, :] == rng384[:, None]).astype(NPBF16)
        cm = {f"oh{r}": np.ascontiguousarray(oh[128 * r : 128 * r + 128]) for r in range(3)}
        in_maps.append({**cm, **shared})

    trace = bool(int(os.environ.get("KERNEL_TRACE", "0")))
    res = run_bass_kernel_spmd(
        nc, in_maps, core_ids=list(range(NCORES)), trace=trace
    )
    LAST_RESULT["exec_time_ns"] = res.exec_time_ns
    LAST_RESULT["trace"] = res.instructions_and_trace

    parts = [res.results[c]["outT"] for c in range(NCORES)]  # each [256, 512]
    out = np.concatenate(parts, axis=0).reshape(BATCH, SEQ, OUT_DIM)
    return np.ascontiguousarray(out.astype(np.float32))


# revision 42
# speedup vs baseline: 1.1029x; 1.0294x over previous
"""CharCNN embedder (ELMo-style) Trainium2 Bass kernel, v3.

Strategy (pure data parallel over 8 cores, 256 tokens each):
  - Embedding lookup as one-hot matmul: one-hot encodings of the char ids
    arrive pre-built from the host (3 x [128, COLS_PAD] bf16, exact 0/1);
    the device runs embT.T @ onehot over 3 row-chunks of the 262-entry
    table -> xT [16, 12800] directly in conv layout. The emb chunks are
    interleaved into the conv stream so they fill PE gaps while the DVE
    drains conv PSUM.
  - im2col by 6 shifted SBUF->SBUF DMA copies -> X7 [112+6, 12800+pad].
    Rows 112..117 are per-position-class indicator rows; the conv weight
    matrix carries -1e30 in those rows for (channel, position) pairs that
    are invalid, so max-over-time needs no masking pass.
  - All 7 convs as one packed [118, 2048] bf16 matmul into 2-bank PSUM
    tiles (16 tokens each); max-over-time as one strided DVE reduce per
    tile (the DVE is the only engine that can drain PSUM with a max; its
    ~226us of reduce work is the kernel's critical path).
  - Highway + projection in token-major form overlapped under the reduce
    stream: layer-0 token-half 0 runs while the conv's second half is
    still reducing. Weight slabs streamed from DRAM in bf16.
"""

import os
import numpy as np
import ml_dtypes

import concourse.bass as bass
import concourse.mybir as mybir
import concourse.tile as tile
from concourse.bass_utils import run_bass_kernel_spmd

F32 = mybir.dt.float32
BF16 = mybir.dt.bfloat16
NPBF16 = ml_dtypes.bfloat16

CNN_OPTIONS = [(1, 32), (2, 32), (3, 64), (4, 128), (5, 256), (6, 512), (7, 1024)]
EMB_DIM = 16
N_CHARS = 262
MAX_CHARS = 50
N_FILTERS = 2048
OUT_DIM = 512
N_HIGHWAY = 2
BATCH, SEQ = 4, 512
NCORES = 8
T_LOC = BATCH * SEQ // NCORES          # 256 tokens per core
COLS = T_LOC * MAX_CHARS               # 12800
COLS_PAD = COLS + 16                   # 12816
KMAX = 7
KROWS = EMB_DIM * KMAX                 # 112
KTOT = KROWS + 6                       # 118 (6 indicator rows for pos 44..49)
NCH = 512                              # xT build chunk width
NXCH = COLS // NCH                     # 25
TOKG = 8                               # tokens per conv matmul
# oc-chunk list: (chunk idx -> kernel size driving its valid-position count)
CHUNK_K = [1, 4, 5, 5, 6, 6, 6, 6, 7, 7, 7, 7, 7, 7, 7, 7]  # m=0 mixed (use 50 pos)
CHUNK_NP = [50 if k == 1 else (MAX_CHARS - k + 1) for k in CHUNK_K]
HWM = 32                               # 4096/128 output chunks per highway layer
KC = 16                                # 2048/128 contraction chunks


def _split_multi_waits(nc):
    """This walrus build encodes at most ONE sync-wait per instruction.
    Hoist extra waits onto dedicated NoOps ahead of the instruction."""
    ctr = [0]
    for f in nc.m.functions:
        for b in f.blocks:
            il = b.instructions
            if not any(
                i.sync_info is not None and len(i.sync_info.on_wait) > 1 for i in il
            ):
                continue
            new = []
            for ins in il:
                si = ins.sync_info
                if si is not None and len(si.on_wait) > 1:
                    waits = list(si.on_wait)
                    for w in waits[:-1]:
                        ctr[0] += 1
                        nop = mybir.InstNoOp(name=f"wsplit-{ctr[0]}", ins=[], outs=[])
                        nop.engine = ins.engine
                        nop.sync_info = mybir.SyncInfo(on_wait=[w], on_update=[])
                        new.append(nop)
                    ins.sync_info = mybir.SyncInfo(
                        on_wait=[waits[-1]], on_update=list(si.on_update)
                    )
                new.append(ins)
            b.instructions = new


def _build_program(split_waits=True):
    nc = bass.Bass(target_bir_lowering=False)

    oh_d = [
        nc.dram_tensor(f"oh{r}", [128, COLS_PAD], BF16, kind="ExternalInput")
        for r in range(3)
    ]
    embt_d = nc.dram_tensor("embt", [384, EMB_DIM], BF16, kind="ExternalInput")
    convw_d = nc.dram_tensor("convw", [KTOT, N_FILTERS], BF16, kind="ExternalInput")
    indic_d = nc.dram_tensor("indic", [6, COLS_PAD], BF16, kind="ExternalInput")
    cbias_d = nc.dram_tensor("cbias", [128, 16], F32, kind="ExternalInput")
    hw0_d = nc.dram_tensor("hw0", [KC, 4, 128, 1024], BF16, kind="ExternalInput")
    hw1_d = nc.dram_tensor("hw1", [KC, 4, 128, 1024], BF16, kind="ExternalInput")
    hb0_d = nc.dram_tensor("hb0", [1, 4096], BF16, kind="ExternalInput")
    hb1_d = nc.dram_tensor("hb1", [1, 4096], BF16, kind="ExternalInput")
    pw_d = nc.dram_tensor("pw", [KC, 128, 512], BF16, kind="ExternalInput")
    pb_d = nc.dram_tensor("pb", [1, 512], BF16, kind="ExternalInput")
    out_d = nc.dram_tensor("outT", [T_LOC, OUT_DIM], F32, kind="ExternalOutput")

    with tile.TileContext(nc) as tc:
        with (
            tc.tile_pool(name="const", bufs=1) as cpool,
            tc.tile_pool(name="elem", bufs=2) as epool,
            tc.tile_pool(name="outp", bufs=2) as outpool,
            tc.tile_pool(name="ps_big", bufs=2, space="PSUM") as ps_big,
            tc.tile_pool(name="ps_xt", bufs=2, space="PSUM") as ps_xt,
            tc.tile_pool(name="ps_hw", bufs=2, space="PSUM") as ps_hw,
        ):
            # one-hot encodings stream per 512-col chunk (3 tiles each);
            # the ring lets the SP run several chunks ahead of the PE
            ohpool_cm = tc.tile_pool(name="ohp", bufs=5)
            ohpool = ohpool_cm.__enter__()

            # ---- constants in ----
            embt_s = cpool.tile([128, 3 * EMB_DIM], BF16, tag="embt")
            for r in range(3):
                nc.sync.dma_start(
                    embt_s[:, 16 * r : 16 * r + 16], embt_d[128 * r : 128 * r + 128, :]
                )
            cbias_s = cpool.tile([128, 16], F32, tag="cbias")
            nc.sync.dma_start(cbias_s[:], cbias_d[:])
            # X7: rows 0-15 xT base, 16-111 shifted copies, 112-117 indicators
            X7 = cpool.tile([KTOT, COLS_PAD], BF16, tag="X7")
            convw_s = cpool.tile([KTOT, N_FILTERS], BF16, tag="convw")
            hb0_s = cpool.tile([1, 4096], BF16, tag="hb0")
            nc.sync.dma_start(hb0_s[:], hb0_d[:])
            hb1_s = cpool.tile([1, 4096], BF16, tag="hb1")
            nc.sync.dma_start(hb1_s[:], hb1_d[:])
            pb_s = cpool.tile([1, 512], BF16, tag="pb")
            nc.sync.dma_start(pb_s[:], pb_d[:])
            onesb_s = cpool.tile([1, 128], BF16, tag="onesb")
            nc.gpsimd.memset(onesb_s[:], 1.0)
            from concourse.masks import make_identity
            ident_s = cpool.tile([128, 128], BF16, tag="ident")
            make_identity(nc, ident_s[:])

            hts = [cpool.tile([128, T_LOC], BF16, tag=f"hT{m}", name=f"hT{m}") for m in range(KC)]
            h1ts = [cpool.tile([128, T_LOC], BF16, tag=f"h1T{m}", name=f"h1T{m}") for m in range(KC)]
            h2ts = [cpool.tile([128, T_LOC], BF16, tag=f"h2T{m}", name=f"h2T{m}") for m in range(KC)]
            htok = [cpool.tile([128, N_FILTERS], BF16, tag=f"htok{t}", name=f"htok{t}") for t in range(2)]
            h1tok = [cpool.tile([128, N_FILTERS], BF16, tag=f"h1tok{t}", name=f"h1tok{t}") for t in range(2)]

            # ---- helpers ----
            def transpose_block(src_ap, dst_ap, nm):
                tp = ps_xt.tile([128, 128], BF16, space="PSUM", tag="xt", name=nm)
                nc.tensor.transpose(tp[:], src_ap, ident_s[:])
                nc.scalar.copy(dst_ap, tp[:])

            # emb chunk: stream 3 one-hot chunk tiles in, 3 accumulated
            # matmuls -> X7 rows 0:16
            def emb_chunk(c):
                c0 = c * NCH
                ohs = []
                for r in range(3):
                    t = ohpool.tile([128, NCH], BF16, tag=f"oh{r}", name=f"oh{r}_{c}")
                    nc.sync.dma_start(t[:], oh_d[r][:, c0 : c0 + NCH])
                    ohs.append(t)
                px = ps_xt.tile([16, NCH], F32, space="PSUM", tag="xt", name=f"px{c}")
                for r in range(3):
                    nc.tensor.matmul(
                        px[:],
                        embt_s[:, 16 * r : 16 * r + 16],
                        ohs[r][:],
                        start=(r == 0),
                        stop=(r == 2),
                    )
                nc.scalar.copy(X7[0:16, c0 : c0 + NCH], px[:])

            def shifts(cl, cu):
                for j in range(1, KMAX):
                    nc.sync.dma_start(
                        X7[16 * j : 16 * j + 16, cl:cu], X7[0:16, cl + j : cu + j]
                    )

            # conv unit: 16 tokens x one 128-chan chunk, one strided reduce
            def conv_unit(nnp, m):
                npos = CHUNK_NP[m]
                ps = ps_big.tile(
                    [128, 1024], F32, space="PSUM", tag="big", name=f"cv{m}_{nnp}"
                )
                for h in range(2):
                    c0 = (2 * nnp + h) * TOKG * MAX_CHARS
                    rhs = (
                        X7[0:KTOT, c0 : c0 + TOKG * MAX_CHARS]
                        .rearrange("p (t c) -> p t c", c=MAX_CHARS)[:, :, 0:npos]
                    )
                    nc.tensor.matmul(
                        ps[:, 512 * h : 512 * h + TOKG * npos],
                        convw_s[:, 128 * m : 128 * m + 128],
                        rhs,
                        start=True,
                        stop=True,
                    )
                ps4 = (
                    ps[:]
                    .rearrange("p (h x) -> p h x", h=2)[:, :, 0 : TOKG * npos]
                    .rearrange("p h (t c) -> p h t c", c=npos)
                )
                dst = hts[m][:, 16 * nnp : 16 * nnp + 16].rearrange(
                    "p (h t) -> p h t", h=2
                )
                nc.vector.reduce_max(dst, ps4, axis=mybir.AxisListType.X)

            def finish_half(half):
                for m in range(16):
                    hsl = hts[m][:, 128 * half : 128 * half + 128]
                    nc.scalar.activation(
                        hsl, hsl, mybir.ActivationFunctionType.Relu,
                        bias=cbias_s[:, m : m + 1], scale=1.0,
                    )
                for c in range(KC):
                    transpose_block(
                        hts[c][:, 128 * half : 128 * half + 128],
                        htok[half][:, 128 * c : 128 * c + 128],
                        f"tpc{half}_{c}",
                    )

            # one DMA per (layer, g, k-half): 8 k-slabs land in a single
            # [128, 8*1024] group tile (8x fewer SP issues and sem hops)
            def hw_grp_dma(gpool, lyr, g, mt):
                w_d = hw0_d if lyr == 0 else hw1_d
                grps = []
                for hk in range(2):
                    grp = gpool.tile(
                        [128, 8 * 1024], BF16, tag="wgrp",
                        name=f"wg{lyr}{g}{mt}{hk}",
                    )
                    nc.sync.dma_start(
                        grp[:].rearrange("p (k x) -> p k x", k=8),
                        w_d[8 * hk : 8 * hk + 8, g].rearrange("k p x -> p k x"),
                    )
                    grps.append(grp)
                return grps

            def hw_chain(lyr, g, bb, mt, grp):
                src_ts = hts if lyr == 0 else h1ts
                h_in = htok if lyr == 0 else h1tok
                hb_s = hb0_s if lyr == 0 else hb1_s
                b = 2 * g + bb
                ps = ps_hw.tile(
                    [128, 512], F32, space="PSUM", tag="hwp",
                    name=f"hwp{lyr}{b}{mt}",
                )
                for k in range(KC):
                    gh = grp[k // 8]
                    o = 1024 * (k % 8) + 512 * bb
                    nc.tensor.matmul(
                        ps[:],
                        src_ts[k][:, 128 * mt : 128 * mt + 128],
                        gh[:, o : o + 512],
                        start=(k == 0), stop=False,
                    )
                nc.tensor.matmul(
                    ps[:], onesb_s[0:1, :],
                    hb_s[0:1, 512 * b : 512 * b + 512],
                    start=False, stop=True,
                )
                r_ = epool.tile([128, 256], BF16, tag="relu", name=f"r{lyr}{b}{mt}")
                nc.scalar.activation(
                    r_[:], ps[:, 0:256], mybir.ActivationFunctionType.Relu
                )
                g_ = epool.tile([128, 256], BF16, tag="gate", name=f"g{lyr}{b}{mt}")
                nc.scalar.activation(
                    g_[:], ps[:, 256:512], mybir.ActivationFunctionType.Sigmoid
                )
                hsl = h_in[mt][:, 256 * b : 256 * b + 256]
                t1 = epool.tile([128, 256], BF16, tag="t1", name=f"t1_{lyr}{b}{mt}")
                nc.vector.tensor_tensor(
                    out=t1[:], in0=hsl, in1=r_[:], op=mybir.AluOpType.subtract
                )
                t2 = epool.tile([128, 256], BF16, tag="t2", name=f"t2_{lyr}{b}{mt}")
                nc.vector.tensor_tensor(
                    out=t2[:], in0=g_[:], in1=t1[:], op=mybir.AluOpType.mult
                )
                if lyr == 0:
                    dst_sl = h1tok[mt][:, 256 * b : 256 * b + 256]
                    nc.vector.tensor_tensor(
                        out=dst_sl, in0=t2[:], in1=r_[:], op=mybir.AluOpType.add
                    )
                    for c in (2 * b, 2 * b + 1):
                        transpose_block(
                            h1tok[mt][:, 128 * c : 128 * c + 128],
                            h1ts[c][:, 128 * mt : 128 * mt + 128],
                            f"tp1_{b}{mt}{c}",
                        )
                else:
                    h2sl = epool.tile(
                        [128, 256], BF16, tag="h2sl", name=f"h2sl{b}{mt}"
                    )
                    nc.vector.tensor_tensor(
                        out=h2sl[:], in0=t2[:], in1=r_[:], op=mybir.AluOpType.add
                    )
                    for ci, c in enumerate((2 * b, 2 * b + 1)):
                        transpose_block(
                            h2sl[:, 128 * ci : 128 * ci + 128],
                            h2ts[c][:, 128 * mt : 128 * mt + 128],
                            f"tp2_{b}{mt}{c}",
                        )

            # ---- pipelined schedule ----
            # emb chunks 0..8 unlock shift block 0 (cols 0..3578, reads to
            # 3583); conv token-half 0 (pairs 0..7) interleaves the remaining
            # emb chunks so the PE fills its reduce-paced gaps. Shift blocks
            # fire as soon as their emb chunks are done. The big const DMAs
            # (conv weights, indicators) are issued after the first few
            # one-hot chunks so the emb pipeline starts immediately.
            for c in range(3):
                emb_chunk(c)
            nc.sync.dma_start(convw_s[:], convw_d[:])
            nc.sync.dma_start(X7[112:118, :], indic_d[:])
            nc.gpsimd.memset(X7[0:16, COLS:COLS_PAD], 0.0)
            emb_chunk(3)
            shifts(0, 1610)
            for c in range(4, 9):
                emb_chunk(c)
            shifts(1610, 3578)
            next_c = [9]
            SH = {13: (3578, 6650), 19: (6650, 9722), 25: (9722, COLS + 6)}

            def pump_emb():
                if next_c[0] >= NXCH:
                    return
                c = next_c[0]
                emb_chunk(c)
                next_c[0] += 1
                if c + 1 in SH:
                    shifts(*SH[c + 1])

            u = 0
            for nnp in range(8):
                for m in range(16):
                    conv_unit(nnp, m)
                    u += 1
                    if u % 8 == 0:
                        pump_emb()
            while next_c[0] < NXCH:
                pump_emb()
            finish_half(0)

            with tc.tile_pool(name="wgrp", bufs=6) as gpool:
                # conv token-half 1 hosts the mt=0 highway work (both layers)
                # in its PE gaps; the mt=1 passes + projection trail it, with
                # their group DMAs streaming early thanks to the 3-deep ring.
                chains = [(0, g, bb) for g in range(4) for bb in range(2)] + [
                    (1, g, bb) for g in range(4) for bb in range(2)
                ]

                grp_cache = {}
                pgrps = []

                def proj_mt(mt):
                    if not pgrps:
                        for hk in range(2):
                            pg = gpool.tile(
                                [128, 8 * 512], BF16, tag="wgrp", name=f"pgrp{hk}"
                            )
                            nc.sync.dma_start(
                                pg[:].rearrange("p (k x) -> p k x", k=8),
                                pw_d[8 * hk : 8 * hk + 8].rearrange("k p x -> p k x"),
                            )
                            pgrps.append(pg)
                    ps = ps_hw.tile(
                        [128, 512], F32, space="PSUM", tag="hwp", name=f"pj{mt}"
                    )
                    for k in range(KC):
                        nc.tensor.matmul(
                            ps[:],
                            h2ts[k][:, 128 * mt : 128 * mt + 128],
                            pgrps[k // 8][:, 512 * (k % 8) : 512 * (k % 8) + 512],
                            start=(k == 0), stop=False,
                        )
                    nc.tensor.matmul(
                        ps[:], onesb_s[0:1, :], pb_s[0:1, :], start=False, stop=True
                    )
                    oc = outpool.tile([128, 512], F32, tag="out", name=f"oc{mt}")
                    nc.scalar.copy(oc[:], ps[:])
                    nc.sync.dma_start(out_d[128 * mt : 128 * mt + 128, :], oc[:])

                def emit_chain(lyr, g, bb, mt):
                    if lyr == "proj":
                        proj_mt(mt)
                        return
                    key = (lyr, g, mt)
                    if key not in grp_cache:
                        grp_cache[key] = hw_grp_dma(gpool, lyr, g, mt)
                    hw_chain(lyr, g, bb, mt, grp_cache[key])

                u = 0
                for nnp in range(8, 16):
                    for m in range(16):
                        conv_unit(nnp, m)
                        u += 1
                        if u % 8 == 0 and chains:
                            emit_chain(*chains.pop(0), 0)
                for lyr, g, bb in chains:
                    emit_chain(lyr, g, bb, 0)
                finish_half(1)
                for g in range(4):
                    for bb in range(2):
                        emit_chain(0, g, bb, 1)
                for g in range(4):
                    for bb in range(2):
                        emit_chain(1, g, bb, 1)
                proj_mt(0)
                proj_mt(1)
            ohpool_cm.__exit__(None, None, None)

    if split_waits:
        _split_multi_waits(nc)
    return nc


def _prep_weights(inputs):
    conv_ws = [np.asarray(inputs[f"conv_w{i}"], np.float32) for i in range(7)]
    conv_bs = [np.asarray(inputs[f"conv_b{i}"], np.float32) for i in range(7)]

    W7 = np.zeros((KTOT, N_FILTERS), np.float32)
    o0 = 0
    for (ksz, oc), w in zip(CNN_OPTIONS, conv_ws):
        for j in range(ksz):
            W7[16 * j : 16 * j + 16, o0 : o0 + oc] = w[:, :, j].T
        # indicator-mask rows: position p = 44+i invalid iff p > 50-ksz
        for i in range(6):
            if (44 + i) > (MAX_CHARS - ksz):
                W7[KROWS + i, o0 : o0 + oc] = -1e30
        o0 += oc

    b_all = np.concatenate(conv_bs)
    cbias = b_all.reshape(16, 128).T.astype(np.float32)

    indic = np.zeros((6, COLS_PAD), np.float32)
    for i in range(6):
        indic[i, (44 + i) : COLS : MAX_CHARS] = 1.0

    emb = np.asarray(inputs["emb"], np.float32)
    embt = np.zeros((384, EMB_DIM), np.float32)
    embt[:N_CHARS] = emb

    # token-form highway: out = h @ w'.T computed as hT-chunks.T @ w'.T-slabs.
    # Channel-interleave so slab b holds [256 nonlin | 256 gate] of block b.
    perm = np.concatenate(
        [
            np.concatenate([np.arange(256 * b, 256 * b + 256),
                            2048 + np.arange(256 * b, 256 * b + 256)])
            for b in range(8)
        ]
    )

    def hw_slabs(w, bvec):
        wp = np.asarray(w, np.float32)[perm]       # [4096, 2048] permuted rows
        bp = np.asarray(bvec, np.float32)[perm]
        wt = wp.T                                   # [2048, 4096]
        arr = wt.reshape(KC, 128, 8, 512).transpose(0, 2, 1, 3)  # [16, 8, 128, 512]
        arr = arr.reshape(KC, 4, 2, 128, 512).transpose(0, 1, 3, 2, 4).reshape(KC, 4, 128, 1024)
        return np.ascontiguousarray(arr).astype(NPBF16), bp[None, :].astype(NPBF16)

    hw0, hb0 = hw_slabs(inputs["hw_w0"], inputs["hw_b0"])
    hw1, hb1 = hw_slabs(inputs["hw_w1"], inputs["hw_b1"])
    pwt = np.asarray(inputs["proj_w"], np.float32).T  # [2048, 512]
    pw = np.ascontiguousarray(pwt.reshape(KC, 128, 512)).astype(NPBF16)
    pb = np.asarray(inputs["proj_b"], np.float32)[None, :].astype(NPBF16)

    return {
        "embt": embt.astype(NPBF16),
        "convw": W7.astype(NPBF16),
        "indic": indic.astype(NPBF16),
        "cbias": cbias,
        "hw0": hw0.astype(NPBF16),
        "hw1": hw1.astype(NPBF16),
        "hb0": hb0,
        "hb1": hb1,
        "pw": pw.astype(NPBF16),
        "pb": pb,
    }


_NC_CACHE = []
LAST_RESULT = {}


def kernel(**inputs) -> np.ndarray:
    if not _NC_CACHE:
        _NC_CACHE.append(_build_program())
    nc = _NC_CACHE[0]

    shared = _prep_weights(inputs)
    ids = np.asarray(inputs["batch_ids"]).astype(np.int64).reshape(-1, MAX_CHARS)
    rng384 = np.arange(384, dtype=np.int64)
    in_maps = []
    for core in range(NCORES):
        flat = ids[core * T_LOC : (core + 1) * T_LOC].reshape(-1)
        oh = np.zeros((384, COLS_PAD), NPBF16)
        oh[:, :COLS] = (flat[None, :] == rng384[:, None]).astype(NPBF16)
        cm = {f"oh{r}": np.ascontiguousarray(oh[128 * r : 128 * r + 128]) for r in range(3)}
        in_maps.append({**cm, **shared})

    trace = bool(int(os.environ.get("KERNEL_TRACE", "0")))
    res = run_bass_kernel_spmd(
        nc, in_maps, core_ids=list(range(NCORES)), trace=trace
    )
    LAST_RESULT["exec_time_ns"] = res.exec_time_ns
    LAST_RESULT["trace"] = res.instructions_and_trace

    parts = [res.results[c]["outT"] for c in range(NCORES)]  # each [256, 512]
    out = np.concatenate(parts, axis=0).reshape(BATCH, SEQ, OUT_DIM)
    return np.ascontiguousarray(out.astype(np.float32))
